# revision 4
# baseline (speedup 1.0000x reference)
"""Chebyshev graph-conv (gnn_message_passing) Trainium2 kernel, v2.

Reference computation:
    x0 = inputs [1,8,V,8,8,8] -> [V, Fin*B*X*Y*Z]
    Chebyshev recurrence with sparse Laplacian (COO, 8 entries/row), K=5
    out = einsum('kvfbxyz,kfo->bovxyz', cheb, weight) + bias

Sharding: dense dim D = Fin*XYZ split over the XYZ axis across 8 cores
(64 spatial positions per core -> local D = 64*8 = 512, laid out d = s*8+f).

v2 design (vs the 357 us baseline):
  - gathers read fp8e3 (e4m3 too lossy, bf16 too slow); stored gens carry
    y_k = 2*x_k so the Chebyshev combine is a plain subtract.
  - spmv: SEL (bf16) stationary x gathered fp8e3 moving, 7 chunks/tile.
  - per-k cheb terms transposed (PE, bf16) into resident SBUF xT buffers;
    the output einsum accumulates over all K in PSUM (k-inner) so only one
    DVE add per (ch,t,h) instead of K.
  - prev generation kept in SBUF (no DRAM round trip); output stored bf16.
"""

import sys

for _p in ("/opt/trn_rl_repo", "/root/.axon_site/_ro/trn_rl_repo"):
    if _p not in sys.path:
        sys.path.append(_p)

import numpy as np

V = 2562
DEG = 8
B, FIN, FOUT, K = 1, 8, 16, 5
XYZ = 512
NCORES = 8
SLOC = XYZ // NCORES  # 64 spatial positions per core
D = SLOC * FIN  # 512 local dense dim, d = s_loc*8 + f

VP = 2688  # V padded to 21*128
NT = VP // 128  # 21 v-tiles
NVCH = 6  # v-chunks of up to 4 v-tiles (5*4 + 1)

DEF_CHUNKS = (7,) * 20 + (1,)

_COMPILED = [None]
LAST_RESULT = [None]


def _build(chunks=DEF_CHUNKS):
    import os as _os

    SIMPLE_OUT = bool(int(_os.environ.get("SIMPLE_OUT", "0")))
    SIMPLE_Y = bool(int(_os.environ.get("SIMPLE_Y", "0")))
    from contextlib import ExitStack

    import concourse.mybir as mybir
    import concourse.tile as tile
    from concourse import bacc

    fp32 = mybir.dt.float32
    i16 = mybir.dt.int16
    bf16 = mybir.dt.bfloat16
    f8 = mybir.dt.float8e3
    f32r = mybir.dt.float32r
    Alu = mybir.AluOpType

    chunks = tuple(chunks)
    NCH = sum(chunks)  # total gather chunks per spmv
    coff = [0]
    for c in chunks:
        coff.append(coff[-1] + c)

    nc = bacc.Bacc(None, target_bir_lowering=False, num_swdge_queues=2)

    x0g = nc.dram_tensor("x0g", [VP, D], bf16, kind="ExternalInput")
    y0g = nc.dram_tensor("y0g", [VP, D], f8, kind="ExternalInput")
    xt0d = nc.dram_tensor("xt0d", [128, 4, VP], bf16, kind="ExternalInput")
    idxd = nc.dram_tensor("idx", [128, NCH * 8], i16, kind="ExternalInput")
    seld = nc.dram_tensor("sel", [128, NCH, 128], bf16, kind="ExternalInput")
    wseld = nc.dram_tensor("wsel", [128, K * 2, 128], bf16, kind="ExternalInput")
    biasd = nc.dram_tensor("biasx", [128, 2], fp32, kind="ExternalInput")
    outd = nc.dram_tensor("outT", [8, 128, VP], bf16, kind="ExternalOutput")

    from concourse.masks import make_identity

    with ExitStack() as ctx:
        tc = ctx.enter_context(tile.TileContext(nc))
        const = ctx.enter_context(tc.tile_pool(name="const", bufs=1))
        xtp = ctx.enter_context(tc.tile_pool(name="xt", bufs=1))
        dram = ctx.enter_context(tc.tile_pool(name="dram", bufs=1, space="DRAM"))
        gp = ctx.enter_context(tc.tile_pool(name="g", bufs=2))
        sp = ctx.enter_context(tc.tile_pool(name="stream", bufs=2))
        ppv = ctx.enter_context(tc.tile_pool(name="psumv", bufs=2, space="PSUM"))
        ppt = ctx.enter_context(tc.tile_pool(name="psumt", bufs=2, space="PSUM"))
        ppo = ctx.enter_context(tc.tile_pool(name="psumo", bufs=4, space="PSUM"))

        SEL = const.tile([128, NCH, 128], bf16)
        IDX = const.tile([128, NCH * 8], i16)
        WSEL = const.tile([128, K * 2, 128], bf16)
        BIA = const.tile([128, 2], fp32)
        IDNB = const.tile([128, 128], bf16)
        IDN = const.tile([128, 128], fp32)

        nc.sync.dma_start(IDX[:], idxd[:])
        nc.sync.dma_start(BIA[:], biasd[:])
        nc.sync.dma_start(SEL[:, NCH - 1 :, :], seld[:, NCH - 1 :, :])
        for q in range(8):
            qs = slice(q * (NCH - 1) // 8, (q + 1) * (NCH - 1) // 8)
            if q == 0:
                nc.sync.dma_start(SEL[:, qs, :], seld[:, qs, :])
            else:
                with tc.tile_wait_until(0.0012 * q):
                    nc.sync.dma_start(SEL[:, qs, :], seld[:, qs, :])
        with tc.tile_wait_until(0.010):
            nc.sync.dma_start(WSEL[:], wseld[:])
        make_identity(nc, IDN[:])
        nc.vector.tensor_copy(IDNB[:], IDN[:])

        # resident cheb-transposed terms xt[k] : [128 (s16,f8)-of-block-t, t, v]
        XT = [const.tile([128, 4, VP], bf16, tag=f"xt{k}", name=f"xt{k}") for k in range(K - 1)]
        # two generations of x in [v, d] layout (tile-major), bf16
        GEN = [xtp.tile([128, NT, D], bf16, tag=f"gen{i}", name=f"gen{i}") for i in range(2)]
        # DRAM gen stores y_k = 2*x_k (fp8e3), gather sources
        yg = [dram.tile([VP, D], f8, tag=f"yg{i}", name=f"yg{i}") for i in range(3)]

        def transpose_tile(get_block):
            """src [128 v, 512 d] bf16 -> psT [128 d-of-block, (4, 128v)] bf16."""
            psT = ppt.tile([128, 512], bf16, tag="psT")
            for t in range(4):
                s = slice(128 * t, 128 * (t + 1))
                nc.tensor.transpose(psT[:, s], get_block(s), IDNB[:])
            return psT

        def out_stage(ch, nvt, xt4):
            """psO[(t,h)] = sum_k WSEL_k,h^T @ xt[k][t, ch] ; +bias -> DRAM."""
            n = 16 if ch == 5 else nvt * 128
            cs = slice(512 * ch, 512 * ch + n)
            ot8 = sp.tile([128, 8, 512], bf16, tag="ot8", bufs=1)
            for ts in ((0, 1), (2, 3)):
                psO = {}
                for h in range(2):
                    for k in range(K):
                        for t in ts:
                            rhs = XT[k][:, t, cs] if k < 4 else xt4[:, t, :n]
                            key = (t, h)
                            if key not in psO:
                                psO[key] = ppo.tile(
                                    [128, 512], fp32, tag="psO", name=f"psO_{t}_{h}"
                                )
                            nc.tensor.matmul(
                                psO[key][:, :n],
                                WSEL[:, k * 2 + h, :],
                                rhs,
                                start=(k == 0),
                                stop=(k == K - 1),
                            )
                for h in range(2):
                    for t in ts:
                        i = t * 2 + h
                        nc.vector.tensor_scalar(
                            ot8[:, i, :n],
                            psO[(t, h)][:, :n],
                            BIA[:, h : h + 1],
                            None,
                            Alu.add,
                        )
            nc.sync.dma_start(
                outd[:, :, cs].rearrange("i p v -> p i v"), ot8[:, :, :n]
            )

        # ---- k = 1..4 (transposes software-pipelined two tiles behind) ----
        from collections import deque

        # estimated phase-end dispatch stamps (ms) for deferred transposes
        PH_END = {1: 0.062, 2: 0.130, 3: 0.200}

        for k in range(1, K):
            if k == 2:
                # x0 -> gen0 (k=2 prev): 3-tile batches, stamped to land
                # during the k=1 phase (the scheduler hoists unstamped DMAs
                # to t=0 where they starve SEL + the first gathers)
                for b in range(7):
                    with tc.tile_wait_until(0.012 + 0.003 * b):
                        nc.scalar.dma_start(
                            GEN[0][:, 3 * b : 3 * b + 3, :],
                            x0g[384 * b : 384 * (b + 1), :].rearrange(
                                "(a p) d -> p a d", p=128
                            ),
                        )
            if k == 3:
                # xt0 (pre-transposed cheb_0) only needed by the out stage
                for q in range(4):
                    with tc.tile_wait_until(0.055 + 0.004 * q):
                        nc.scalar.dma_start(XT[0][:, q, :], xt0d[:, q, :])
            srcg = y0g if k == 1 else yg[k - 2]
            pendq = deque()  # delayed (get_block, vt, xt4) transposes
            yq = None

            def flush(pend, k=None):
                if pend is None:
                    return
                get_block, vt, xt4 = pend
                psT = transpose_tile(get_block)
                if k < 4:
                    nc.vector.tensor_copy(
                        XT[k][:, :, 128 * vt : 128 * (vt + 1)],
                        psT[:].rearrange("p (t v) -> p t v", t=4),
                    )
                else:
                    ch, vl = vt // 4, vt % 4
                    nvt = 4 if ch < 5 else 1
                    nc.vector.tensor_copy(
                        xt4[:, :, 128 * vl : 128 * (vl + 1)],
                        psT[:].rearrange("p (t v) -> p t v", t=4),
                    )
                    if vl == nvt - 1:
                        out_stage(ch, nvt, xt4)

            xt4 = None
            g = None
            order = [NT - 1] + list(range(NT - 1))
            for oi, vt in enumerate(order):
                nch = chunks[vt]
                o = coff[vt]
                psV = ppv.tile([128, 512], fp32, tag="psV", bufs=2)
                # single-tile gather (>1024 idxs breaks the HW DGE path)
                g = gp.tile([128, chunks[0], D], f8, tag="g", bufs=4)
                goff = 0
                with tc.high_priority():
                    nc.gpsimd.dma_gather(
                        g[:, :nch, :],
                        srcg[:],
                        IDX[:, o * 8 : (o + nch) * 8],
                        num_idxs=nch * 128,
                        num_idxs_reg=nch * 128,
                        elem_size=D,
                        queue_num=oi % 2,
                    )
                if k < 4:
                    for j in range(nch):
                        nc.tensor.matmul(
                            psV[:],
                            SEL[:, o + j, :],
                            g[:, goff + j, :],
                            start=(j == 0),
                            stop=(j == nch - 1),
                        )
                else:
                    # transposed spmv: gathered chunk slices as stationary,
                    # SEL as moving -> psV holds T(2 L x3) directly, so the
                    # combine runs in transposed space against resident XT[2]
                    # (no PE transposes, no psT copy in the saturated tail)
                    for db in range(4):
                        ds = slice(128 * db, 128 * (db + 1))
                        for j in range(nch):
                            nc.tensor.matmul(
                                psV[:, ds],
                                g[:, goff + j, ds],
                                SEL[:, o + j, :],
                                start=(j == 0),
                                stop=(j == nch - 1),
                                skip_group_check=True,
                            )
                # k=4 keeps per-tile (2-back) flush so the out stage
                # interleaves with the spmv; k<4 defers all transposes to the
                # phase tail so the mm stream is never blocked behind a
                # transpose waiting on its combine.
                if k == 4 and len(pendq) >= 2:
                    flush(pendq.popleft(), k)
                elif k < 4 and len(pendq) >= 2:
                    # dispatch-stamped past the estimated phase end so the
                    # scheduler cannot wedge it (combine-dependent) between
                    # this phase's matmul bursts
                    with tc.tile_wait_until(PH_END[k]):
                        flush(pendq.popleft(), k)
                # combine
                if k == 1:
                    nc.vector.tensor_scalar(
                        GEN[1][:, vt, :], psV[:], 0.5, None, Alu.mult
                    )
                    get_block = lambda s, vt=vt: GEN[1][:, vt, s]
                    xv_ap = GEN[1][:, vt, :]
                elif k < 4:
                    nc.vector.tensor_tensor(
                        GEN[k % 2][:, vt, :], psV[:], GEN[k % 2][:, vt, :], Alu.subtract
                    )
                    get_block = lambda s, vt=vt, k=k: GEN[k % 2][:, vt, s]
                    xv_ap = GEN[k % 2][:, vt, :]
                else:
                    ch4, vl4 = vt // 4, vt % 4
                    if vl4 == 0:
                        xt4 = sp.tile([128, 4, 512], bf16, tag="xt4", bufs=2)
                    nc.vector.tensor_tensor(
                        xt4[:, :, 128 * vl4 : 128 * (vl4 + 1)],
                        psV[:].rearrange("p (t v) -> p t v", t=4),
                        XT[2][:, :, 128 * vt : 128 * (vt + 1)],
                        Alu.subtract,
                    )
                    nvt4 = 4 if ch4 < 5 else 1
                    if vl4 == nvt4 - 1:
                        out_stage(ch4, nvt4, xt4)
                    get_block = None
                    xv_ap = None
                if k < 4:
                    # quantize y_k = 2*x_k -> fp8 into a batched store buffer;
                    # batches follow processing order with contiguous v rows
                    if vt == NT - 1:
                        bstart, blen, bslot = NT - 1, 1, 0
                    else:
                        bstart = 3 * (vt // 3)
                        blen = min(3, (NT - 1) - bstart)
                        bslot = vt - bstart
                    if bslot == 0:
                        yq = sp.tile([128, 3, D], f8, tag="yq", bufs=2)
                    nc.scalar.mul(yq[:, bslot, :], xv_ap, 2.0)
                    if bslot == blen - 1:
                        eng = nc.sync if (bstart // 3) % 2 == 0 else nc.scalar
                        if SIMPLE_Y:
                            for a in range(blen):
                                eng.dma_start(
                                    yg[k - 1][
                                        128 * (bstart + a) : 128 * (bstart + a + 1), :
                                    ],
                                    yq[:, a, :],
                                )
                        else:
                            eng.dma_start(
                                yg[k - 1][
                                    128 * bstart : 128 * (bstart + blen), :
                                ].rearrange("(a p) d -> p a d", p=128),
                                yq[:, :blen, :],
                            )
                if get_block is not None:
                    pendq.append((get_block, vt, xt4))
            while pendq:
                if k < 4:
                    with tc.tile_wait_until(PH_END[k]):
                        flush(pendq.popleft(), k)
                else:
                    flush(pendq.popleft(), k)

    nc.compile()
    return nc


def _host_prep(inputs, lap_rows, lap_cols, lap_vals, weight, bias):
    inputs = np.asarray(inputs, dtype=np.float32)
    lap_rows = np.asarray(lap_rows)
    lap_cols = np.asarray(lap_cols)
    lap_vals = np.asarray(lap_vals, dtype=np.float32)
    weight = np.asarray(weight, dtype=np.float32)
    bias = np.asarray(bias, dtype=np.float32)
    import ml_dtypes

    EPAD = VP * DEG
    nnz = lap_rows.shape[0]
    order = np.argsort(lap_rows, kind="stable")
    srows = lap_rows[order]
    assert np.array_equal(
        np.repeat(np.arange(V, dtype=srows.dtype), DEG), srows
    ), "expected exactly DEG entries per row"
    e_cols = np.zeros(EPAD, np.int64)
    e_vals = np.zeros(EPAD, np.float32)
    e_cols[:nnz] = lap_cols[order]
    e_vals[:nnz] = lap_vals[order]

    # per-v-tile dedup: gather each unique col once; SEL folds vals and
    # scatters every (unique col -> output v) pair of the tile
    uniq = [np.unique(e_cols[1024 * vt : 1024 * (vt + 1)]) for vt in range(NT)]
    chunks = tuple(max(1, (len(u) + 127) // 128) for u in uniq)
    NCH = sum(chunks)
    coff = np.concatenate([[0], np.cumsum(chunks)]).astype(int)

    idx_np = np.zeros((128, NCH * 8), np.int16)
    sel_np = np.zeros((128, NCH, 128), np.float32)
    for vt in range(NT):
        u = uniq[vt]
        nch = chunks[vt]
        o = coff[vt]
        slots = np.zeros(nch * 128, np.int64)
        slots[: len(u)] = u
        w = slots.reshape(nch * 8, 16).T.astype(np.int16)  # wrapped-16
        idx_np[:, o * 8 : (o + nch) * 8] = np.tile(w, (8, 1))
        col2slot = np.zeros(V + 1, np.int64)
        col2slot[u] = np.arange(len(u))
        ecols = e_cols[1024 * vt : 1024 * (vt + 1)]
        evals = e_vals[1024 * vt : 1024 * (vt + 1)]
        sl = col2slot[ecols]
        m = np.arange(1024) // DEG  # output row within v-tile
        np.add.at(sel_np, (sl % 128, o + sl // 128, m), evals)
    sel_np = sel_np.astype(ml_dtypes.bfloat16)

    # output-stage weight selection: rows p=s_loc*8+f, cols q=s_loc*8+o
    wsel_np = np.zeros((128, K * 2, 128), np.float32)
    sl = np.arange(16)
    for k in range(K):
        for h in range(2):
            for f in range(FIN):
                for o in range(8):
                    wsel_np[sl * 8 + f, k * 2 + h, sl * 8 + o] = weight[k, f, 8 * h + o]

    bias_np = np.zeros((128, 2), np.float32)
    p = np.arange(128)
    for h in range(2):
        bias_np[p, h] = bias[8 * h + p % 8]

    # x0 shards: [V, s, f] per core
    xt = inputs.reshape(FIN, V, XYZ).transpose(1, 2, 0)  # [V, 512, 8]
    x0s = []
    xt0s = []
    for m in range(NCORES):
        x0m = np.zeros((VP, D), np.float32)
        x0m[:V] = xt[:, SLOC * m : SLOC * (m + 1), :].reshape(V, D)
        x0s.append(x0m)
        # pre-transposed xt0: [128 (s16,f8)-of-block-t, t, v]
        xt0s.append(
            np.ascontiguousarray(
                x0m.reshape(VP, 4, 128).transpose(2, 1, 0)
            )
        )
    return x0s, xt0s, idx_np, sel_np, wsel_np, bias_np, chunks


def kernel(inputs, lap_rows, lap_cols, lap_vals, weight, bias):
    import ml_dtypes as _ml

    from concourse.bass_utils import run_bass_kernel_spmd

    x0s, xt0s, idx_np, sel_np, wsel_np, bias_np, chunks = _host_prep(
        inputs, lap_rows, lap_cols, lap_vals, weight, bias
    )

    if _COMPILED[0] is None or _COMPILED[0][0] != chunks:
        _COMPILED[0] = (chunks, _build(chunks))
    nc = _COMPILED[0][1]

    in_maps = [
        {
            "x0g": x0s[m].astype(_ml.bfloat16),
            "y0g": (2.0 * x0s[m]).astype(_ml.float8_e3m4),
            "xt0d": xt0s[m].astype(_ml.bfloat16),
            "idx": idx_np,
            "sel": sel_np,
            "wsel": wsel_np.astype(_ml.bfloat16),
            "biasx": bias_np,
        }
        for m in range(NCORES)
    ]
    import os

    trace = bool(int(os.environ.get("KERNEL_TRACE", "0")))
    res = run_bass_kernel_spmd(
        nc, in_maps, core_ids=list(range(NCORES)), trace=trace
    )
    LAST_RESULT[0] = res

    # unshard: outT [8=(t,h), 128=(s_loc,o_loc), VP] per core
    parts = []
    for m in range(NCORES):
        r = np.asarray(res.results[m]["outT"]).astype(np.float32)  # [8, 128, VP]
        r = r.reshape(4, 2, 16, 8, VP)[:, :, :, :, :V]  # [t, h, sl, ol, v]
        # o = 8h + ol ; s_local_in_core = 16t + sl
        r = r.transpose(1, 3, 4, 0, 2).reshape(FOUT, V, SLOC)  # [o, v, s]
        parts.append(r)
    out = np.concatenate(parts, axis=2)  # [o, v, 512]
    return np.ascontiguousarray(
        out.reshape(1, FOUT, V, 8, 8, 8).astype(np.float32)
    )
